# revision 19
# baseline (speedup 1.0000x reference)
"""Trainium2 Bass kernel for a dense transformer block (B=2, T=2048, C=1024, nh=16, H=4096).

Strategy (8 NeuronCores, no device collectives):

  Launch 1 (head-parallel): cores 0-3 <- batch 0, cores 4-7 <- batch 1; each core
    handles 4 attention heads over the full sequence. All matmul operands are
    bf16 (fp32 PSUM accumulate). LN1 statistics are computed with ones-vector
    matmuls into a [16,128] PSUM tile (one partition row per 128-token tile), so
    the row math (mu, rstd) runs 16-partition-parallel on tiny tiles. x is then
    normalized in-place (2 DVE ops per feature tile) and QKV are plain matmuls
    with the LN affine folded into the weights on host. Causal attention uses
    512-token query chunks, key-tile loop software-pipelined; exp on ScalarE
    (1024-wide tiles to amortize the pipeline fill), diagonal multiplicative
    masks; softmax division: DVE reciprocal of the PSUM denominator row ->
    GpSimd partition broadcast -> one DVE multiply (no DMA round trips).
    c_proj partials are computed per chunk-pair, interleaved with the next
    pair's attention. Output: row-parallel c_proj partial [2048, 1024] bf16.

  Host: pure re-slicing of the partials (no arithmetic).

  Launch 2 (token-parallel): each core takes a 512-token slice: sums the 4 proj
    partials on-device (indicator matmuls), + residual + proj_b -> LN2 (DVE
    bn_stats, Rsqrt on ScalarE) -> c_fc (bf16, feature-major hidden)
    -> gaussian activation (2 ScalarE passes; mu/sigma/fc_b folded into the
    activation bias/scale, gamma/beta folded into fc2 weights/bias on host)
    -> c_fc2 (bf16) -> + residual -> final output slice (fp32). MLP weights
    stream from HBM as bf16 in contiguous per-chunk layouts.

Both launches are uniform SPMD programs run via run_bass_kernel_spmd.
"""

import hashlib
import os
import shutil
from contextlib import ExitStack

import ml_dtypes
import numpy as np

import concourse.bass as bass
import concourse.tile as tile
from concourse import bacc, mybir
from concourse.bass_utils import run_bass_kernel_spmd

F32 = mybir.dt.float32
BF16 = mybir.dt.bfloat16
NPBF16 = np.dtype(ml_dtypes.bfloat16)
AF = mybir.ActivationFunctionType
ALU = mybir.AluOpType

N_CORES = 8
T = 2048          # tokens per batch
C = 1024          # model dim
NH_LOC = 4        # heads per core (launch 1)
HS = 64           # head size
HID = 4096        # mlp hidden
TS = 512          # tokens per core (launch 2)

LAST_EXEC_NS = {}  # launch name -> exec_time_ns (filled when tracing enabled)

_CACHE_DIR = "/tmp/neff_cache"


def _install_compile_cache():
    import concourse.bass2jax as b2j

    if getattr(b2j, "_neff_cache_installed", False):
        return
    real = b2j.compile_bir_kernel

    def cached(bir_json, tmpdir, neff_name="file.neff"):
        os.makedirs(_CACHE_DIR, exist_ok=True)
        h = hashlib.sha256(bir_json).hexdigest()
        cpath = os.path.join(_CACHE_DIR, h + ".neff")
        out = os.path.join(tmpdir, neff_name)
        if os.path.exists(cpath):
            shutil.copyfile(cpath, out)
            return out
        res = real(bir_json, tmpdir, neff_name)
        shutil.copyfile(res, cpath)
        return res

    b2j.compile_bir_kernel = cached
    b2j._neff_cache_installed = True


# --------------------------------------------------------------------------
# Launch 1: LN1 + QKV + causal attention (4 heads) + c_proj partial
# --------------------------------------------------------------------------
def build_l1():
    nc = bacc.Bacc("TRN2", target_bir_lowering=False, debug=False,
                   num_devices=N_CORES)
    xT_d = nc.dram_tensor("xT", [4, 128, 8, 512], BF16, kind="ExternalInput")
    wqk_d = nc.dram_tensor("wqkT", [128, 8, 512], BF16, kind="ExternalInput")
    wv_d = nc.dram_tensor("wvT", [128, 8, 256], BF16, kind="ExternalInput")
    bqk_d = nc.dram_tensor("bqk", [128, 4], F32, kind="ExternalInput")
    bv_d = nc.dram_tensor("bv", [1, 256], BF16, kind="ExternalInput")
    pw_d = nc.dram_tensor("projwT", [128, 2, 1024], BF16, kind="ExternalInput")
    mask_d = nc.dram_tensor("masks", [128, 4, 512], BF16, kind="ExternalInput")
    yp_d = nc.dram_tensor("yp", [T, C], BF16, kind="ExternalOutput")

    n_ttiles = T // 128          # 16
    n_tc = T // 512              # 4 attention query chunks

    with tile.TileContext(nc) as tc, ExitStack() as ctx:
        consts = ctx.enter_context(tc.tile_pool(name="consts", bufs=1))
        ones1 = consts.tile([128, 1], BF16)
        nc.vector.memset(ones1[:], 1.0)
        onesr = consts.tile([1, 128], BF16)
        nc.vector.memset(onesr[:], 1.0)
        eps1 = consts.tile([1, 1], F32)
        nc.vector.memset(eps1[:], 1e-5)
        bqk_sb = consts.tile([128, 4], F32)
        nc.sync.dma_start(out=bqk_sb[:], in_=bqk_d[:])
        bv_sb = consts.tile([1, 256], BF16)
        nc.sync.dma_start(out=bv_sb[:], in_=bv_d[:])

        big = ctx.enter_context(tc.tile_pool(name="big", bufs=1))
        slab = big.tile([128, 4, 8, 512], BF16)     # raw x, all 4 chunks
        qkT = big.tile([128, 4, T], BF16)           # Q feats (0,1), K feats (2,3)
        # AV stationary: col 0 = ones (denominator -> psum partition 0),
        # cols 32:96 = V (y -> psum partitions 32:96, a legal DVE offset)
        v_sb = big.tile([128, n_ttiles, NH_LOC, 128], BF16)
        yT = big.tile([128, 2, T], BF16)
        nc.vector.memset(v_sb[:, :, :, 0:1], 1.0)
        nc.vector.memset(v_sb[:, :, :, 1:64], 0.0)

        wpool = ctx.enter_context(tc.tile_pool(name="wpool", bufs=1))
        wqk_sb = wpool.tile([128, 8, 512], BF16)
        nc.sync.dma_start(out=wqk_sb[:], in_=wqk_d[:])
        wv_sb = wpool.tile([128, 8, 256], BF16)
        nc.sync.dma_start(out=wv_sb[:], in_=wv_d[:])

        # ---- P1: LN1 stats + normalize + QKV, software-pipelined per chunk --
        with tc.tile_pool(name="p1sq", bufs=3) as p1sq, \
             tc.tile_pool(name="p1r", bufs=2) as p1r, \
             tc.tile_pool(name="pbc", bufs=2) as pbc, \
             tc.tile_pool(name="pxn", bufs=2) as pxn, \
             tc.tile_pool(name="pstat", bufs=2, space="PSUM") as pstat, \
             tc.tile_pool(name="p2p", bufs=2, space="PSUM") as p2p:
            for tch in range(4):
                nc.sync.dma_start(out=slab[:, tch], in_=xT_d[tch])

            def emit_stats(tch):
                ps_s = pstat.tile([1, 512], F32, tag="s", name=f"ps_s{tch}")
                ps_q = pstat.tile([1, 512], F32, tag="q", name=f"ps_q{tch}")
                for c in range(8):
                    sq = p1sq.tile([128, 512], BF16, tag="sq")
                    nc.gpsimd.tensor_mul(sq[:], slab[:, tch, c, :],
                                         slab[:, tch, c, :])
                    nc.tensor.matmul(ps_s[:], ones1[:], slab[:, tch, c, :],
                                     start=(c == 0), stop=(c == 7),
                                     skip_group_check=True)
                    nc.tensor.matmul(ps_q[:], ones1[:], sq[:],
                                     start=(c == 0), stop=(c == 7),
                                     skip_group_check=True)
                return ps_s, ps_q

            cur = emit_stats(0)
            for tch in range(4):
                nxt = emit_stats(tch + 1) if tch < 3 else None
                ps_s, ps_q = cur
                sl = slice(tch * 512, (tch + 1) * 512)
                # row math: nmu=-s/C; var=q/C-nmu^2; r=1/sqrt(var+eps); nrmu=r*nmu
                nmu = p1r.tile([1, 512], F32, tag="nmu")
                nc.vector.tensor_scalar(out=nmu[:], in0=ps_s[:],
                                        scalar1=-1.0 / C, scalar2=None,
                                        op0=ALU.mult)
                msq = p1r.tile([1, 512], F32, tag="msq")
                nc.vector.tensor_mul(msq[:], nmu[:], nmu[:])
                dv = p1r.tile([1, 512], F32, tag="dv")
                nc.vector.scalar_tensor_tensor(out=dv[:], in0=ps_q[:],
                                               scalar=1.0 / C, in1=msq[:],
                                               op0=ALU.mult, op1=ALU.subtract)
                sdr = p1r.tile([1, 512], F32, tag="sd")
                nc.scalar.activation(out=sdr[:], in_=dv[:], func=AF.Sqrt,
                                     bias=eps1[0:1], scale=1.0)
                rrow = p1r.tile([1, 512], F32, tag="r")
                nc.vector.reciprocal_approx_fast(rrow[:], sdr[:])
                rrow_h = p1r.tile([1, 512], BF16, tag="rh")
                nc.vector.tensor_copy(rrow_h[:], rrow[:])
                nrmu = p1r.tile([1, 512], BF16, tag="nrmu")
                nc.vector.tensor_mul(nrmu[:], rrow[:], nmu[:])
                rb = pbc.tile([128, 512], BF16, tag="rb")
                nc.gpsimd.partition_broadcast(rb[:], rrow_h[:])
                nrb = pbc.tile([128, 512], BF16, tag="nrb")
                nc.gpsimd.partition_broadcast(nrb[:], nrmu[:])
                xn = pxn.tile([128, 8, 512], BF16, tag="xn")
                for c in range(8):
                    nc.vector.tensor_mul(xn[:, c, :], slab[:, tch, c, :], rb[:])
                    nc.vector.tensor_add(xn[:, c, :], xn[:, c, :], nrb[:])
                for f in range(4):
                    ps = p2p.tile([128, 512], F32, tag="qk")
                    for c in range(8):
                        nc.tensor.matmul(
                            ps[:], wqk_sb[:, c, f * 128:(f + 1) * 128],
                            xn[:, c, :], start=(c == 0), stop=(c == 7))
                    nc.vector.tensor_scalar(out=qkT[:, f, sl], in0=ps[:],
                                            scalar1=bqk_sb[:, f:f + 1],
                                            scalar2=None, op0=ALU.add)
                for j in range(4):
                    tt = tch * 4 + j
                    js = slice(j * 128, (j + 1) * 128)
                    psv = p2p.tile([128, 256], F32, tag="v")
                    nc.tensor.matmul(psv[:], onesr[:], bv_sb[:],
                                     start=True, stop=False,
                                     skip_group_check=True)
                    for c in range(8):
                        nc.tensor.matmul(psv[:], xn[:, c, js], wv_sb[:, c, :],
                                         start=False, stop=(c == 7),
                                         skip_group_check=True)
                    nc.vector.tensor_copy(
                        v_sb[:, tt, :, 64:128],
                        psv[:].rearrange("p (h d) -> p h d", h=NH_LOC))
                cur = nxt

        late = ctx.enter_context(tc.tile_pool(name="late", bufs=1))
        pw_sb = late.tile([128, 2, 1024], BF16)
        nc.sync.dma_start(out=pw_sb[:], in_=pw_d[:])
        mask_sb = late.tile([128, 4, 512], BF16)
        nc.sync.dma_start(out=mask_sb[:], in_=mask_d[:])

        # ---- P3: attention. QK for a head pair is packed into concurrent
        # row-group matmuls. Head A's AV+softmax-divide runs inline (2-gen
        # score pipeline keeps ScalarE exp saturated); head B's at tiles are
        # kept in SBUF and its AV runs as a dense PE-only pass afterwards. ----
        with tc.tile_pool(name="p3a", bufs=1) as p3a, \
             tc.tile_pool(name="p3s", bufs=4) as p3s, \
             tc.tile_pool(name="p3sc", bufs=1, space="PSUM") as p3sc, \
             tc.tile_pool(name="p3py", bufs=1, space="PSUM") as p3py:

            def divide(pyt, hh, tcx):
                qsl = slice(tcx * 512, (tcx + 1) * 512)
                po = (hh % 2) * 64
                rrow = p3s.tile([1, 512], F32, tag="rr")
                nc.vector.reciprocal_approx_fast(rrow[:], pyt[0:1, :])
                db = p3s.tile([64, 512], F32, tag="db")
                nc.gpsimd.partition_broadcast(db[:], rrow[:])
                nc.vector.tensor_mul(yT[po:po + 64, hh // 2, qsl],
                                     pyt[64:128, :], db[:])

            for pair in ((0, 1), (2, 3)):
                smax = 4 * pair[1] + 4
                for hp in range(2):
                    hA, hB = 2 * hp, 2 * hp + 1
                    qf, kf = hp, 2 + hp
                    pysA = {tcx: p3py.tile([128, 512], F32,
                                           tag=f"py{tcx % 2}",
                                           name=f"pyA{hp}_{tcx}_{pair[0]}")
                            for tcx in pair}

                    def emit_qk(s):
                        tcs = [tcx for tcx in pair if s <= 4 * tcx + 3]
                        if not tcs:
                            return [], []
                        width = 1024 if len(tcs) == 2 else 512
                        ksl = slice(s * 128, (s + 1) * 128)
                        sA = p3sc.tile([128, 1024], F32, tag="scA", bufs=2,
                                       name=f"scA{hp}_{s}_{pair[0]}")
                        sB = p3sc.tile([128, 1024], F32, tag="scB", bufs=1,
                                       name=f"scB{hp}_{s}_{pair[0]}")
                        for i, tcx in enumerate(tcs):
                            qsl = slice(tcx * 512, (tcx + 1) * 512)
                            nc.tensor.matmul(
                                sA[:, i * 512:(i + 1) * 512],
                                qkT[0:64, kf, ksl], qkT[0:64, qf, qsl],
                                start=True, stop=True, skip_group_check=True)
                            nc.tensor.matmul(
                                sB[:, i * 512:(i + 1) * 512],
                                qkT[64:128, kf, ksl], qkT[64:128, qf, qsl],
                                start=True, stop=True, skip_group_check=True)
                        atA = p3a.tile([128, 1024], BF16, tag="atA", bufs=3,
                                       name=f"atA{hp}_{s}_{pair[0]}")
                        atB = p3a.tile([128, 1024], BF16, tag="atB", bufs=16,
                                       name=f"atB{hp}_{s}_{pair[0]}")
                        nc.scalar.activation(out=atA[:, 0:width],
                                             in_=sA[:, 0:width], func=AF.Exp)
                        nc.scalar.activation(out=atB[:, 0:width],
                                             in_=sB[:, 0:width], func=AF.Exp)
                        tA, tB = [], []
                        for i, tcx in enumerate(tcs):
                            csl = slice(i * 512, (i + 1) * 512)
                            if tcx == s // 4:
                                nc.vector.tensor_mul(atA[:, csl], atA[:, csl],
                                                     mask_sb[:, s % 4, :])
                                nc.vector.tensor_mul(atB[:, csl], atB[:, csl],
                                                     mask_sb[:, s % 4, :])
                            tA.append((tcx, atA[:, csl]))
                            tB.append((tcx, atB[:, csl]))
                        return tA, tB

                    savedB = []
                    cur = emit_qk(0)
                    for s in range(smax):
                        nxt = emit_qk(s + 1) if s + 1 < smax else ([], [])
                        tA, tB = cur
                        savedB.append((s, tB))
                        for tcx, atv in tA:
                            nc.tensor.matmul(pysA[tcx][:], v_sb[:, s, hA, :],
                                             atv, start=(s == 0),
                                             stop=(s == 4 * tcx + 3),
                                             skip_group_check=True)
                            if s == 4 * tcx + 3:
                                divide(pysA[tcx], hA, tcx)
                        cur = nxt
                    # head B: dense AV pass from the saved at tiles
                    pysB = {tcx: p3py.tile([128, 512], F32,
                                           tag=f"py{tcx % 2}",
                                           name=f"pyB{hp}_{tcx}_{pair[0]}")
                            for tcx in pair}
                    for s, tB in savedB:
                        for tcx, atv in tB:
                            nc.tensor.matmul(pysB[tcx][:], v_sb[:, s, hB, :],
                                             atv, start=(s == 0),
                                             stop=(s == 4 * tcx + 3),
                                             skip_group_check=True)
                            if s == 4 * tcx + 3:
                                divide(pysB[tcx], hB, tcx)

        # ---- P4: c_proj partial ----
        with tc.tile_pool(name="p4o", bufs=3) as p4o, \
             tc.tile_pool(name="p4ps", bufs=2, space="PSUM") as p4p:
            for tt in range(n_ttiles):
                for co in range(2):
                    pp = p4p.tile([128, 512], F32)
                    for cl in range(2):
                        nc.tensor.matmul(
                            pp[:], yT[:, cl, tt * 128:(tt + 1) * 128],
                            pw_sb[:, cl, co * 512:(co + 1) * 512],
                            start=(cl == 0), stop=(cl == 1))
                    ot = p4o.tile([128, 512], BF16)
                    nc.vector.tensor_copy(ot[:], pp[:])
                    nc.sync.dma_start(
                        out=yp_d[tt * 128:(tt + 1) * 128,
                                 co * 512:(co + 1) * 512], in_=ot[:])
    nc.compile()
    return nc


# --------------------------------------------------------------------------
# Launch 2: reduce partials + residual + LN2 + MLP + residual
# --------------------------------------------------------------------------
def build_l2(s_act: float):
    nc = bacc.Bacc("TRN2", target_bir_lowering=False, debug=False,
                   num_devices=N_CORES)
    yp4_d = nc.dram_tensor("yp4", [16, 128, C], BF16, kind="ExternalInput")
    gones_d = nc.dram_tensor("gones", [128, 4, 128], BF16, kind="ExternalInput")
    xs_d = nc.dram_tensor("xs", [TS, C], F32, kind="ExternalInput")
    pb_d = nc.dram_tensor("pb", [1, C], F32, kind="ExternalInput")
    fb2_d = nc.dram_tensor("fb2", [1, C], F32, kind="ExternalInput")
    ab_d = nc.dram_tensor("abias", [128, 32], F32, kind="ExternalInput")
    fcw_d = nc.dram_tensor("fcwT", [8, 128, 8, 512], BF16, kind="ExternalInput")
    fc2w_d = nc.dram_tensor("fc2wT", [8, 128, 4, C], BF16, kind="ExternalInput")
    id_d = nc.dram_tensor("ident", [128, 128], BF16, kind="ExternalInput")
    out_d = nc.dram_tensor("out", [TS, C], F32, kind="ExternalOutput")

    n_ttiles = TS // 128    # 4

    with tile.TileContext(nc) as tc, ExitStack() as ctx:
        consts = ctx.enter_context(tc.tile_pool(name="consts", bufs=1))
        ident = consts.tile([128, 128], BF16)
        nc.sync.dma_start(out=ident[:], in_=id_d[:])
        eps_sb = consts.tile([128, 1], F32)
        nc.vector.memset(eps_sb[:], 1e-5)
        pb_row = consts.tile([1, C], F32)
        nc.sync.dma_start(out=pb_row[:], in_=pb_d[:])
        pb_b = consts.tile([128, C], F32)
        nc.gpsimd.partition_broadcast(pb_b[:], pb_row[:])
        fb2_row = consts.tile([1, C], F32)
        nc.sync.dma_start(out=fb2_row[:], in_=fb2_d[:])
        fb2_b = consts.tile([128, C], F32)
        nc.gpsimd.partition_broadcast(fb2_b[:], fb2_row[:])
        ab_sb = consts.tile([128, 32], F32)
        nc.sync.dma_start(out=ab_sb[:], in_=ab_d[:])
        gones_sb = consts.tile([128, 4, 128], BF16)
        nc.sync.dma_start(out=gones_sb[:], in_=gones_d[:])

        big = ctx.enter_context(tc.tile_pool(name="big", bufs=1))
        h2T = big.tile([128, 8, TS], BF16)         # 8KB/p
        x2pb = big.tile([128, n_ttiles, C], F32)   # x2 + fc2 bias, 16KB/p
        actT = big.tile([128, 32, TS], BF16)       # 32KB/p
        yp4all = big.tile([128, 16, C], BF16)      # 32KB/p
        xsall = big.tile([128, n_ttiles, C], F32)  # 16KB/p
        for i in range(16):
            nc.sync.dma_start(out=yp4all[:, i, :], in_=yp4_d[i])
        for tt in range(n_ttiles):
            nc.sync.dma_start(out=xsall[:, tt, :],
                              in_=xs_d[tt * 128:(tt + 1) * 128, :])

        # ---- P1: reduce partials, LN2, transpose ----
        with tc.tile_pool(name="q1", bufs=3) as q1, \
             tc.tile_pool(name="q1s", bufs=4) as q1s, \
             tc.tile_pool(name="q1psum", bufs=2, space="PSUM") as q1p:
            for tt in range(n_ttiles):
                x2 = q1.tile([128, C], F32, tag="x2")
                px2 = [q1p.tile([128, 512], F32, tag=f"px{ch}", name=f"px{tt}_{ch}")
                       for ch in range(2)]
                for j in range(4):
                    for ch in range(2):
                        nc.tensor.matmul(
                            px2[ch][:], gones_sb[:, j, :],
                            yp4all[:, tt * 4 + j, ch * 512:(ch + 1) * 512],
                            start=(j == 0), stop=(j == 3),
                            skip_group_check=True)
                xpb = q1.tile([128, C], F32, tag="xpb")
                nc.gpsimd.tensor_add(xpb[:], xsall[:, tt, :], pb_b[:])
                for ch in range(2):
                    csl = slice(ch * 512, (ch + 1) * 512)
                    nc.vector.tensor_add(x2[:, csl], px2[ch][:], xpb[:, csl])
                nc.vector.tensor_add(x2pb[:, tt, :], x2[:], fb2_b[:])
                stats = q1s.tile([128, 2, 6], F32)
                x2g = x2[:].rearrange("p (g d) -> p g d", g=2)
                nc.vector.bn_stats(out=stats[:, 0, :], in_=x2g[:, 0, :])
                nc.vector.bn_stats(out=stats[:, 1, :], in_=x2g[:, 1, :])
                mv = q1s.tile([128, 2], F32)
                nc.vector.bn_aggr(out=mv[:], in_=stats[:])
                sd = q1s.tile([128, 1], F32, tag="sd")
                nc.scalar.activation(out=sd[:], in_=mv[:, 1:2], func=AF.Sqrt,
                                     bias=eps_sb[:], scale=1.0)
                rstd = q1s.tile([128, 1], F32)
                nc.vector.reciprocal(rstd[:], sd[:])
                h2 = q1.tile([128, C], BF16, tag="h2")
                nc.vector.tensor_scalar(out=h2[:], in0=x2[:],
                                        scalar1=mv[:, 0:1], scalar2=rstd[:],
                                        op0=ALU.subtract, op1=ALU.mult)
                for c in range(8):
                    pt = q1p.tile([128, 128], BF16, tag="pt")
                    nc.tensor.transpose(pt[:], h2[:, c * 128:(c + 1) * 128], ident[:])
                    nc.vector.tensor_copy(h2T[:, c, tt * 128:(tt + 1) * 128], pt[:])

        # ---- P2: c_fc + gaussian activation (feature-major) ----
        with tc.tile_pool(name="q2w", bufs=2) as q2w, \
             tc.tile_pool(name="q2t", bufs=3) as q2t, \
             tc.tile_pool(name="q2psum", bufs=3, space="PSUM") as q2p:
            for hc in range(8):
                wt = q2w.tile([128, 8, 512], BF16)
                nc.sync.dma_start(out=wt[:], in_=fcw_d[hc])
                for ht in range(4):
                    pu = q2p.tile([128, TS], F32)
                    for c in range(8):
                        nc.tensor.matmul(
                            pu[:], wt[:, c, ht * 128:(ht + 1) * 128],
                            h2T[:, c, :], start=(c == 0), stop=(c == 7))
                    hi = hc * 4 + ht
                    usq = q2t.tile([128, TS], F32)
                    nc.scalar.activation(out=usq[:], in_=pu[:], func=AF.Square,
                                         bias=ab_sb[:, hi:hi + 1], scale=s_act)
                    nc.scalar.activation(out=actT[:, hi, :], in_=usq[:],
                                         func=AF.Exp, scale=-1.0)

        # ---- P3: c_fc2 + residual ----
        with tc.tile_pool(name="q3w", bufs=2) as q3w, \
             tc.tile_pool(name="q3o", bufs=3) as q3o, \
             tc.tile_pool(name="q3psum", bufs=1, space="PSUM") as q3p:
            po_tiles = []
            for tt in range(n_ttiles):
                row = []
                for co in range(2):
                    po_t = q3p.tile([128, 512], F32, tag=f"o{tt}{co}",
                                    name=f"po{tt}{co}")
                    row.append(po_t)
                po_tiles.append(row)
            for kr in range(8):
                w2 = q3w.tile([128, 4, C], BF16)
                nc.sync.dma_start(out=w2[:], in_=fc2w_d[kr])
                for tt in range(n_ttiles):
                    for k4 in range(4):
                        k = kr * 4 + k4
                        for co in range(2):
                            nc.tensor.matmul(
                                po_tiles[tt][co][:],
                                actT[:, k, tt * 128:(tt + 1) * 128],
                                w2[:, k4, co * 512:(co + 1) * 512],
                                start=(kr == 0 and k4 == 0),
                                stop=(kr == 7 and k4 == 3),
                                skip_group_check=True)
            for tt in range(n_ttiles):
                for co in range(2):
                    ot = q3o.tile([128, 512], F32)
                    nc.vector.tensor_add(ot[:], po_tiles[tt][co][:],
                                         x2pb[:, tt, co * 512:(co + 1) * 512])
                    nc.sync.dma_start(
                        out=out_d[tt * 128:(tt + 1) * 128,
                                  co * 512:(co + 1) * 512], in_=ot[:])
    nc.compile()
    return nc


# --------------------------------------------------------------------------
# Host-side orchestration
# --------------------------------------------------------------------------
_PROG_CACHE = {}


def _get_prog(key, builder, *args):
    if key not in _PROG_CACHE:
        _PROG_CACHE[key] = builder(*args)
    return _PROG_CACHE[key]


def _causal_masks4():
    s = np.arange(128)[:, None]
    t = np.arange(512)[None, :]
    ms = [((s + 128 * m) <= t).astype(np.float32) for m in range(4)]
    return np.ascontiguousarray(np.stack(ms, axis=1))  # [128, 4, 512]


def _perm(w, tiles, width):
    """[tiles*128, width] -> [128, tiles, width] (partition-major for DMA)."""
    return np.ascontiguousarray(w.reshape(tiles, 128, width).transpose(1, 0, 2))


def _bf(a):
    return np.ascontiguousarray(np.asarray(a).astype(NPBF16))


def kernel(x, ln1_w, ln1_b, attn_w, attn_b, proj_w, proj_b,
           ln2_w, ln2_b, fc_w, fc_b, fc2_w, fc2_b,
           mu, sigma, gamma, beta, n_head):
    x = np.asarray(x, dtype=np.float32)
    attn_w = np.asarray(attn_w, dtype=np.float32)
    attn_b = np.asarray(attn_b, dtype=np.float32)
    proj_w = np.asarray(proj_w, dtype=np.float32)
    proj_b = np.asarray(proj_b, dtype=np.float32)
    fc_w = np.asarray(fc_w, dtype=np.float32)
    fc_b = np.asarray(fc_b, dtype=np.float32)
    fc2_w = np.asarray(fc2_w, dtype=np.float32)
    fc2_b = np.asarray(fc2_b, dtype=np.float32)
    ln1_w = np.asarray(ln1_w, dtype=np.float32)
    ln1_b = np.asarray(ln1_b, dtype=np.float32)
    ln2_w = np.asarray(ln2_w, dtype=np.float32)
    ln2_b = np.asarray(ln2_b, dtype=np.float32)
    mu = float(mu)
    sigma = float(sigma)
    gamma = float(gamma)
    beta = float(beta)
    n_head = int(n_head)

    B = x.shape[0]
    assert x.shape == (B, T, C) and B == 2 and n_head == 16

    _install_compile_cache()
    trace = bool(int(os.environ.get("BASS_KERNEL_TRACE", "0")))

    sig = abs(sigma) + 1e-8
    s_act = float(1.0 / (np.sqrt(2.0) * sig))

    # Fold LN affine params into the consuming projection weights (host-side).
    attn_w_eff = attn_w * ln1_w[None, :]
    attn_b_eff = attn_b + attn_w @ ln1_b
    fc_w_eff = fc_w * ln2_w[None, :]
    fc_b_eff = fc_b + fc_w @ ln2_b

    # ---- launch 1 ----
    nc1 = _get_prog(("l1",), build_l1)
    masks = _bf(_causal_masks4())
    xb = [None, None]
    for b in range(B):
        xTb = _perm(np.ascontiguousarray(x[b].T.astype(NPBF16)), 8, T)
        xb[b] = np.ascontiguousarray(
            xTb.reshape(128, 8, 4, 512).transpose(2, 0, 1, 3))
    in_maps1 = []
    for c in range(N_CORES):
        b, hg = c // 4, c % 4
        q_rows = attn_w_eff[hg * 256:(hg + 1) * 256] * 0.125
        k_rows = attn_w_eff[C + hg * 256:C + (hg + 1) * 256]
        v_rows = attn_w_eff[2 * C + hg * 256:2 * C + (hg + 1) * 256]
        wqk = np.concatenate([q_rows, k_rows], axis=0)   # [512, 1024]
        bqk = np.concatenate([attn_b_eff[hg * 256:(hg + 1) * 256] * 0.125,
                              attn_b_eff[C + hg * 256:C + (hg + 1) * 256]])
        bv = attn_b_eff[2 * C + hg * 256:2 * C + (hg + 1) * 256]
        m = {
            "xT": xb[b],
            "wqkT": _bf(_perm(np.ascontiguousarray(wqk.T), 8, 512)),
            "wvT": _bf(_perm(np.ascontiguousarray(v_rows.T), 8, 256)),
            "bqk": np.ascontiguousarray(bqk.reshape(4, 128).T),
            "bv": _bf(bv[None, :]),
            "projwT": _bf(_perm(
                np.ascontiguousarray(proj_w[:, hg * 256:(hg + 1) * 256].T), 2, 1024)),
            "masks": masks,
        }
        in_maps1.append(m)
    res1 = run_bass_kernel_spmd(nc1, in_maps1, list(range(N_CORES)), trace=trace)
    if res1.exec_time_ns is not None:
        LAST_EXEC_NS["l1"] = res1.exec_time_ns
    yps = [res1.results[c]["yp"] for c in range(N_CORES)]

    # ---- launch 2 ----
    nc2 = _get_prog(("l2", s_act), build_l2, s_act)
    fc2w_eff = (gamma * fc2_w).T                        # [4096, 1024]
    fb2_eff = fc2_b + beta * fc2_w.sum(axis=1)
    abias = ((fc_b_eff - mu) * s_act).reshape(32, 128).T    # [128, 32]
    fcwT_p = _perm(np.ascontiguousarray(fc_w_eff.T), 8, HID)      # [128, 8, 4096]
    fcw_chunks = _bf(
        fcwT_p.reshape(128, 8, 8, 512).transpose(2, 0, 1, 3))   # [8,128,8,512]
    fc2wT_p = _perm(np.ascontiguousarray(fc2w_eff), 32, C)      # [128, 32, 1024]
    fc2w_chunks = _bf(
        fc2wT_p.reshape(128, 8, 4, C).transpose(1, 0, 2, 3))    # [8,128,4,1024]
    ident = _bf(np.eye(128, dtype=np.float32))
    p = np.arange(128)
    gones = np.zeros((128, 4, 128), dtype=np.float32)
    for j in range(4):
        gones[p, j, 32 * j + (p % 32)] = 1.0
    gones = _bf(gones)
    in_maps2 = []
    for c in range(N_CORES):
        b, sl = c // 4, c % 4
        t0 = sl * TS
        yp4 = np.stack([np.asarray(yps[b * 4 + g])[t0:t0 + TS] for g in range(4)])
        # interleave the 4 partials into 32-token groups: [16, 4*32, C]
        yp4s = np.ascontiguousarray(
            yp4.reshape(4, 16, 32, C).transpose(1, 0, 2, 3).reshape(16, 128, C))
        m = {
            "yp4": yp4s,
            "xs": np.ascontiguousarray(x[b, t0:t0 + TS]),
            "pb": proj_b[None, :],
            "fb2": np.ascontiguousarray(fb2_eff[None, :]),
            "abias": np.ascontiguousarray(abias),
            "gones": gones,
            "fcwT": fcw_chunks,
            "fc2wT": fc2w_chunks,
            "ident": ident,
        }
        in_maps2.append(m)
    res2 = run_bass_kernel_spmd(nc2, in_maps2, list(range(N_CORES)), trace=trace)
    if res2.exec_time_ns is not None:
        LAST_EXEC_NS["l2"] = res2.exec_time_ns

    out = np.empty((B, T, C), dtype=np.float32)
    for c in range(N_CORES):
        b, sl = c // 4, c % 4
        out[b, sl * TS:(sl + 1) * TS] = res2.results[c]["out"]
    return out
